# revision 23
# baseline (speedup 1.0000x reference)
"""NVFP4 BlackwellLinear kernel for 8 Trainium2 NeuronCores.

Strategy (column-parallel, per sharding hint):
  - weight_q/weight_scale/bias are sharded along out_features (16384 -> 8 x 2048).
  - Weights are prepacked on host: w_deq = weight_q * weight_scale (exact in bf16,
    <= 6 significand bits), shipped pre-transposed as wt[K, N_loc] bf16.
  - x is replicated; each core quantizes the full activation tensor on-device,
    then does the bf16 matmul out^T = w_deq @ x_deq^T with bias fused into the
    PSUM->SBUF eviction. Host transposes/concats the per-core out^T slices.

Engine assignment is by dependency chain to avoid head-of-line blocking on the
strict-FIFO engine queues:
  - gpsimd (SWDGE): x loads, per-block reciprocal, dequant multiply, xq stores
  - vector: amax reduce, scale ops, the three custom fp4-rounding passes
  - sync (SP HWDGE): transpose DMAs only
  - scalar (ACT HWDGE): weight loads, PSUM evictions + bias, output stores
  - tensor: matmuls only

fp4 round-to-nearest per element:
  v2  = clamp(x * (2/s), +-12)                       [Q1 custom DVE]
  qh  = (v2 + sign_binade(v2)*0.25) & 0xFFC00000     [Q3A custom DVE]
  q2  = qh*qh >= 16 ? qh : (v2 + 1.5*2^23) - 1.5*2^23  [QSEL custom DVE, RNE fused]
  xq  = q2 * (s/2)                                   [gpsimd tensor_tensor]
which matches the reference grid exactly except at exact ties (measure-zero).
"""

import numpy as np

TOK = 4096
K = 4096
OUT_F = 16384
N_CORES = 8
NL = OUT_F // N_CORES  # 2048
P = 128
BLOCK = 16

# tunables
CHUNK = 256           # matmul token-chunk (rhs free dim)
CHUNKS = (128, 128) + (256,) * 15
QS = 1024             # quant compute slice (free elems)
XT_SLOTS = 64         # xT tile slots ([P, CHUNK] bf16 each)
XDMA = 1024           # x load granularity (free elems)
LEAD = 512            # quant token lead ahead of matmul consumer
PSUM_BUFS = 8

MAGIC = 12582912.0    # 1.5 * 2^23
FP8_MIN = 2.0 ** -9

_REGISTERED = {}


def _register_ops():
    """Register the custom DVE ops (idempotent). shas computed dynamically."""
    if _REGISTERED:
        return _REGISTERED
    import concourse.dve_ops as dve_ops
    from concourse.dve_ops import DveOp
    from concourse.dve_spec import (
        Spec, Src0, Src1, C0, C1, C2, Zero, lower, AluOp, Bin,
        maxx, minn, select, _has_src1,
    )
    from concourse.dve_uop import DveOpSpec

    def ref_q1(in0, in1, s0, s1, imm2):
        a = np.asarray(in0, np.float32)
        b = np.asarray(in1, np.float32).reshape(a.shape)
        return np.clip((a * b).astype(np.float32), np.float32(-s0), np.float32(s0))

    body_q1 = minn(maxx(Src0 * Src1, Zero - C0), C0)
    spec_q1 = Spec(body=body_q1, reference=ref_q1)

    def ref_q3a(in0, in1, s0, s1, imm2):
        v2 = np.asarray(in0, np.float32)
        p = (v2.view(np.uint32) & np.uint32(0xFF800000)).view(np.float32)
        bh = (v2 + p * np.float32(imm2)).astype(np.float32)
        return (bh.view(np.uint32) & np.uint32(0xFFC00000)).view(np.float32)

    # trunc-to-1-mantissa-bit without NaN-pattern masks (NaN sign is mangled
    # on the f32 read path): bh & 0xFFC00000 == (bh & -inf) | (bh & 0x00400000)
    p3 = Bin(AluOp.BITWISE_AND, Src0, C0)  # C0 = -inf mask AP (0xFF800000)
    bh3 = Src0 + p3 * C2
    q3a_hi = Bin(AluOp.BITWISE_AND, bh3, C0)
    q3a_lo = Bin(AluOp.BITWISE_AND, bh3, C1)  # C1 = 0x00400000 subnormal mask AP
    spec_q3a = Spec(body=Bin(AluOp.BITWISE_OR, q3a_hi, q3a_lo), reference=ref_q3a)

    def ref_qsel(in0, in1, s0, s1, imm2):
        qh = np.asarray(in0, np.float32)
        v2 = np.asarray(in1, np.float32)
        m = ((v2 + np.float32(s0)).astype(np.float32) - np.float32(s0)).astype(
            np.float32)
        return np.where(qh * qh >= np.float32(imm2), qh, m).astype(np.float32)

    # q2 = select(qh^2 >= 16, qh, rne_int(v2)) — the RNE-to-integer path
    # (magic-number round) fused with the branch select in one DVE pass.
    spec_qsel = Spec(body=select(Src0 * Src0 >= C2, Src0, (Src1 + C0) - C0),
                     reference=ref_qsel)

    def mk(name, spec):
        shas = {}
        for ver in ("v3", "v4"):
            uops = lower(spec, ver=ver)
            row = dve_ops._CUSTOM_DVE_ROW_BASE + len(dve_ops.OPS)
            dos = DveOpSpec(name=name, opcode=row, uops=uops, rd1_en=_has_src1(spec))
            shas[ver] = dos.sha(ver)
        op = DveOp(name, spec, subdim=False, uops_sha=shas)
        dve_ops.OPS.append(op)
        dve_ops.CUSTOM_DVE_SPECS[name] = spec
        dve_ops._SUB_OPCODE_FOR_NAME[name] = dve_ops._CUSTOM_DVE_ROW_BASE + len(dve_ops.OPS) - 1
        return op

    _REGISTERED["Q1"] = mk("NVFP4_MULCLAMP_ANT", spec_q1)
    _REGISTERED["Q3A"] = mk("NVFP4_TRUNC1_ANT", spec_q3a)
    _REGISTERED["QSEL"] = mk("NVFP4_SELRNE_ANT", spec_qsel)
    return _REGISTERED


_NC_CACHE = {}


def build_nc(tok=TOK, k=K, nl=NL, chunk=CHUNK, qs=QS, xt_slots=XT_SLOTS,
             xdma=XDMA, lead=LEAD, psum_bufs=PSUM_BUFS, chunks=None,
             debug_xdeq=False):
    if chunks is None:
        chunks = CHUNKS
    chunks = tuple(chunks)
    assert sum(chunks) == tok
    key = (tok, k, nl, chunk, qs, xt_slots, xdma, lead, psum_bufs, chunks,
           debug_xdeq)
    if key in _NC_CACHE:
        return _NC_CACHE[key]

    import concourse.bass as bass
    import concourse.mybir as mybir
    import concourse.tile as tile
    from concourse.tile_rust import add_dep_helper
    from concourse import bacc

    ops = _register_ops()
    dt = mybir.dt

    KT = k // P            # k-tiles (32)
    NT = nl // P           # n-tiles (16)
    nblk = qs // BLOCK     # 16-blocks per quant slice

    nc = bacc.Bacc("TRN2", target_bir_lowering=False, debug=False,
                   num_devices=N_CORES)

    x_d = nc.dram_tensor("x", [tok, k], dt.float32, kind="ExternalInput").ap()
    wt_d = nc.dram_tensor("wt", [k, nl], dt.bfloat16, kind="ExternalInput").ap()
    b_d = nc.dram_tensor("bias", [nl, 1], dt.float32, kind="ExternalInput").ap()
    o_d = nc.dram_tensor("outT", [nl, tok], dt.float32, kind="ExternalOutput").ap()
    xq_d = nc.dram_tensor("xdeq", [tok, k], dt.bfloat16,
                          kind="ExternalOutput" if debug_xdeq else "Internal").ap()

    with tile.TileContext(nc) as tc:
        with (
            tc.tile_pool(name="const", bufs=1) as constp,
            tc.tile_pool(name="wres", bufs=1) as wres,
            tc.tile_pool(name="xin", bufs=4) as xin,
            tc.tile_pool(name="scal", bufs=4) as scal,
            tc.tile_pool(name="scr", bufs=2) as scrp,
            tc.tile_pool(name="v2p", bufs=2) as v2p,
            tc.tile_pool(name="qhp", bufs=2) as qhp,
            tc.tile_pool(name="q2p", bufs=4) as q2p,
            tc.tile_pool(name="xtp", bufs=xt_slots) as xtp,
            tc.tile_pool(name="outp", bufs=2) as outp,
            tc.tile_pool(name="psum", bufs=psum_bufs, space="PSUM") as psump,
        ):
            # ---- constants ----
            nmask = constp.tile([P, 1], dt.float32, tag="nmask")
            nc.vector._memset_packed(nmask[:], 0xFF800000)
            smask = constp.tile([P, 1], dt.float32, tag="smask")
            nc.vector._memset_packed(smask[:], 0x00400000)
            bias_t = constp.tile([P, NT], dt.float32, tag="bias")
            for n in range(NT):
                nc.scalar.dma_start(bias_t[:, n:n + 1], b_d[n * P:(n + 1) * P, :])

            # ---- resident weights (ACT ring; consumed in kk order) ----
            wt_tiles = []
            for kk in range(KT):
                t = wres.tile([P, nl], dt.bfloat16, tag=f"wt{kk}")
                nc.scalar.dma_start(t[:], wt_d[kk * P:(kk + 1) * P, :])
                wt_tiles.append(t)

            nblk_x = xdma // BLOCK  # 16-blocks per x-load slab (128)
            nsl_x = xdma // qs      # quant slices per x-load slab

            # one-slice-deferred dequant ops: keeps the gpsimd queue head
            # always-satisfied (its mult's QSEL finished a full slice ago),
            # so x loads queued behind it never stall.
            pending = []
            store_insts = {}  # (m, xdma-slab) -> xq store instruction

            def flush_pending(keep=0):
                while len(pending) > keep:
                    pending.pop(0)()

            def quant_mtile(m):
                rows = slice(m * P, (m + 1) * P)
                for h in range(k // xdma):
                    h0 = h * xdma
                    xsl = xin.tile([P, xdma], dt.float32, tag="xsl")
                    nc.gpsimd.dma_start(xsl[:], x_d[rows, h0:h0 + xdma])
                    # per-16-block scales, batched across the load slab
                    amax = scal.tile([P, nblk_x], dt.float32, tag="amax")
                    nc.vector.tensor_reduce(
                        amax[:], xsl[:].rearrange("p (b s) -> p b s", s=BLOCK),
                        axis=mybir.AxisListType.X, op=mybir.AluOpType.max,
                        apply_absolute_value=True)
                    s8 = scal.tile([P, nblk_x], dt.float8e4, tag="s8")
                    nc.vector.tensor_scalar(
                        out=s8[:], in0=amax[:], scalar1=1.0 / 6.0, scalar2=None,
                        op0=mybir.AluOpType.mult)
                    sh = scal.tile([P, nblk_x], dt.float32, tag="sh")
                    nc.vector.tensor_scalar(
                        out=sh[:], in0=s8[:], scalar1=FP8_MIN, scalar2=0.5,
                        op0=mybir.AluOpType.max, op1=mybir.AluOpType.mult)
                    # r2 = 1/sh = 2/s (correctly-rounded reciprocal)
                    r2 = scal.tile([P, nblk_x], dt.float32, tag="r2")
                    rs = scrp.tile([P, nblk_x], dt.float32, tag="rs")
                    nc.vector.reciprocal_approx_accurate(r2[:], sh[:], rs[:])
                    for s in range(nsl_x):
                        c0 = s * qs
                        b0 = c0 // BLOCK
                        shb = sh[:, b0:b0 + nblk].unsqueeze(2).to_broadcast(
                            (P, nblk, BLOCK))
                        r2b = r2[:, b0:b0 + nblk].unsqueeze(2).to_broadcast(
                            (P, nblk, BLOCK))
                        # v2 = clamp(x * 2/s, +-12)
                        v2 = v2p.tile([P, qs], dt.float32, tag="v2")
                        nc.vector._custom_dve(
                            ops["Q1"], out=v2[:], in0=xsl[:, c0:c0 + qs],
                            in1=r2b, s0=12.0)
                        # qh = trunc-to-1-mantissa-bit(v2 + binade/4)
                        qh = qhp.tile([P, qs], dt.float32, tag="qh")
                        nc.vector._custom_dve(
                            ops["Q3A"], out=qh[:], in0=v2[:],
                            s0=nmask[:, :], s1=smask[:, :], imm2=0.25)
                        # q2 = select(qh^2>=16, qh, rne(v2)) -> bf16
                        q2 = q2p.tile([P, qs], dt.bfloat16, tag="q2")
                        nc.vector._custom_dve(
                            ops["QSEL"], out=q2[:], in0=qh[:], in1=v2[:],
                            s0=MAGIC, imm2=16.0)
                        flush_pending(keep=1)

                        def dequant(q2=q2, shb=shb, rows=rows, d0=h0 + c0,
                                    m=m, h=h, s=s):
                            # xdeq = q2 * s/2, in place, then store
                            q2b = q2[:].rearrange("p (b s) -> p b s", s=BLOCK)
                            nc.gpsimd.tensor_tensor(
                                out=q2b, in0=q2b, in1=shb,
                                op=mybir.AluOpType.mult)
                            si = nc.gpsimd.dma_start(
                                xq_d[rows, d0:d0 + qs], q2[:])
                            si = si.ins if hasattr(si, "ins") else si
                            store_insts[(m, (h * xdma + s * qs) // qs)] = si
                        pending.append(dequant)

            def matmul_chunk(t0, ck):
                xts = []
                for kk in range(KT):
                    xt = xtp.tile([P, chunk], dt.bfloat16, tag="xt",
                                  name="xt")[:, :ck]
                    eng = nc.sync
                    ti = eng.dma_start_transpose(
                        xt, xq_d[t0:t0 + ck, kk * P:(kk + 1) * P])
                    ti = ti.ins if hasattr(ti, "ins") else ti
                    sl = (kk * P) // qs
                    for m in range(t0 // P, (t0 + ck + P - 1) // P):
                        add_dep_helper(ti, store_insts[(m, sl)],
                                       reason="xq dram raw")
                    xts.append(xt)
                for n in range(NT):
                    ps = psump.tile([P, chunk], dt.float32, tag="ps",
                                    name="ps")[:, :ck]
                    for kk in range(KT):
                        nc.tensor.matmul(
                            ps, wt_tiles[kk][:, n * P:(n + 1) * P], xts[kk],
                            start=(kk == 0), stop=(kk == KT - 1))
                    ob = outp.tile([P, chunk], dt.float32, tag="ob",
                                   name="ob")[:, :ck]
                    nc.scalar.activation(
                        ob, ps, mybir.ActivationFunctionType.Identity,
                        bias=bias_t[:, n:n + 1], scale=1.0)
                    nc.scalar.dma_start(
                        o_d[n * P:(n + 1) * P, t0:t0 + ck], ob)

            t0 = 0
            mdone = 0
            for ck in chunks:
                t0n = t0 + ck
                target = min(tok, t0n + lead)
                while mdone * P < target:
                    quant_mtile(mdone)
                    mdone += 1
                if mdone * P >= tok:
                    flush_pending()
                matmul_chunk(t0, ck)
                t0 = t0n

    nc.compile()
    _NC_CACHE[key] = nc
    return nc


def _prep_weights(weight_q, weight_scale, bias):
    """Host prepack: per-core transposed dequantized bf16 weights."""
    import ml_dtypes
    wq = np.asarray(weight_q, np.float32).reshape(OUT_F, K // BLOCK, BLOCK)
    ws = np.asarray(weight_scale, np.float32)[:, :, None]
    wdeq = (wq * ws).reshape(OUT_F, K)  # exact: <=6 significand bits
    wts, biases = [], []
    for c in range(N_CORES):
        sl = wdeq[c * NL:(c + 1) * NL]          # [NL, K]
        wts.append(np.ascontiguousarray(sl.T).astype(ml_dtypes.bfloat16))
        biases.append(np.ascontiguousarray(
            np.asarray(bias, np.float32)[c * NL:(c + 1) * NL].reshape(NL, 1)))
    return wts, biases


def kernel(x, weight_q, weight_scale, bias):
    from concourse.bass_utils import run_bass_kernel_spmd

    nc = build_nc()
    x2 = np.ascontiguousarray(np.asarray(x, np.float32).reshape(TOK, K))
    wts, biases = _prep_weights(weight_q, weight_scale, bias)
    in_maps = [{"x": x2, "wt": wts[c], "bias": biases[c]} for c in range(N_CORES)]
    res = run_bass_kernel_spmd(nc, in_maps, list(range(N_CORES)))
    out = np.empty((TOK, OUT_F), np.float32)
    for c in range(N_CORES):
        out[:, c * NL:(c + 1) * NL] = res.results[c]["outT"].T
    return out.reshape(1, TOK, OUT_F)


if __name__ == "__main__":
    rng = np.random.default_rng(0)
    x = rng.normal(size=(1, TOK, K)).astype(np.float32)
    wq = rng.normal(size=(OUT_F, K)).astype(np.float32)
    ws = rng.random(size=(OUT_F, K // BLOCK)).astype(np.float32) + 0.1
    b = rng.normal(size=(OUT_F,)).astype(np.float32)
    out = kernel(x, wq, ws, b)
    print(out.shape, out.dtype)
